# revision 6
# baseline (speedup 1.0000x reference)
# Causal self-attention (B=2, T=2048, D=1024, H=16, HD=64) with RoPE on 8 TRN2
# cores. Data-parallel over batch (2 groups of 4 cores), tensor-parallel over
# heads within a group (4 heads = 2 pairs per core).
#
# Schedule (single pass, engines balanced):
#  - x arrives t-chunk-major ([kt, 512-col chunk] DMAs) so the qkv^T projection
#    pipelines behind the x load instead of stalling on it.
#  - q/k projected per head-pair into PSUM, RoPE'd (DVE+Pool) into bf16 SBUF;
#    v projected directly in [t, hd] layout (x chunk as the stationary
#    operand), so no PE transposes are needed for AV.
#  - attention strip si (512 q's) runs as soon as its q/k/v t-chunks exist,
#    interleaved into the remaining projection matmuls as PE filler: S^T
#    (bf16) -> exp on the Scalar engine (only exp lives there) -> causal
#    triangle mask (Pool affine_select / DVE mask-multiply) -> AV with an
#    augmented ones-column producing the softmax denominator.
#  - out-projection (row-sharded, partial [D, T] per core) per strip, woven
#    into the last strip's exp gaps; host sums 4 partials per batch.
import sys
import os

sys.path.insert(0, "/opt/trn_rl_repo")

import numpy as np
import ml_dtypes

import concourse.bass as bass  # noqa: F401
import concourse.mybir as mybir
from concourse import bacc
from concourse.tile import TileContext
from concourse.bass_utils import run_bass_kernel_spmd
from contextlib import ExitStack

F32 = mybir.dt.float32
F32R = mybir.dt.float32r
BF16 = mybir.dt.bfloat16
AF = mybir.ActivationFunctionType
ALU = mybir.AluOpType

B, T, D = 2, 2048, 1024
H, HD = 16, 64
NCORES = 8
GROUPS = NCORES // B          # cores per batch = 4
HPC = H // GROUPS             # heads per core = 4
NK = D // 128                 # contraction tiles = 8
NTC = T // 512                # t-chunks = 4
SCALE = HD ** -0.5

# hd interleave: new row 2j <- orig j, new row 2j+1 <- orig j+32 so the
# rotate-half partner of every row is its neighbour (swappable by a 32-lane
# stream shuffle).
PI = np.empty(HD, dtype=np.int64)
PI[0::2] = np.arange(32)
PI[1::2] = np.arange(32, 64)

SWAP_MASK = []
for _i in range(16):
    SWAP_MASK += [2 * _i + 1, 2 * _i]


def _build_program():
    nc = bacc.Bacc("TRN2", target_bir_lowering=False, debug=False,
                   num_devices=NCORES)
    d_xT = nc.dram_tensor("xT", [D, T], F32, kind="ExternalInput").ap()
    # cols: q01|k01|q23|k23 (PI-interleaved, 128 each) then v0..v3 (plain, 256)
    d_w = nc.dram_tensor("w_cat", [D, 6 * 128], F32, kind="ExternalInput").ap()
    d_wo = nc.dram_tensor("w_o", [2 * 128, D], BF16, kind="ExternalInput").ap()
    d_cos = nc.dram_tensor("cos2", [128, T], BF16, kind="ExternalInput").ap()
    d_sin = nc.dram_tensor("sin2", [128, T], BF16, kind="ExternalInput").ap()
    d_tri = nc.dram_tensor("tri", [128, 128], BF16, kind="ExternalInput").ap()
    d_out = nc.dram_tensor("outp", [D, T], BF16, kind="ExternalOutput").ap()
    dbg = bool(int(os.environ.get("KDEBUG", "0")))
    if dbg:
        d_dbg_q0 = nc.dram_tensor("dbg_q0", [128, T], BF16,
                                  kind="ExternalOutput").ap()
        d_dbg_k0 = nc.dram_tensor("dbg_k0", [128, T], BF16,
                                  kind="ExternalOutput").ap()
        d_dbg_va0 = nc.dram_tensor("dbg_va0", [128, 16 * 130], BF16,
                                   kind="ExternalOutput").ap()
        d_dbg_o0 = nc.dram_tensor("dbg_o0", [128, T], BF16,
                                  kind="ExternalOutput").ap()

    with TileContext(nc) as tc, nc.allow_low_precision(reason="bf16 attn"):
        with ExitStack() as root:
            xp = root.enter_context(tc.tile_pool(name="xp", bufs=1))
            wp = root.enter_context(tc.tile_pool(name="wp", bufs=1))
            tab = root.enter_context(tc.tile_pool(name="tab", bufs=1))
            qkp = root.enter_context(tc.tile_pool(name="qkp", bufs=1))
            vap_p = root.enter_context(tc.tile_pool(name="vap", bufs=1))
            otp = root.enter_context(tc.tile_pool(name="otp", bufs=1))
            wop = root.enter_context(tc.tile_pool(name="wop", bufs=1))
            rsc = root.enter_context(tc.tile_pool(name="rsc", bufs=3))
            ptp = root.enter_context(tc.tile_pool(name="ptp", bufs=6))
            rp = root.enter_context(tc.tile_pool(name="rp", bufs=2))
            fop = root.enter_context(tc.tile_pool(name="fop", bufs=4))

            x_sb = [xp.tile([128, T], F32R, tag=f"x{kt}", name=f"xsb{kt}")
                    for kt in range(NK)]
            w_sb = [wp.tile([128, 6 * 128], F32R, tag=f"w{kt}",
                            name=f"wsb{kt}") for kt in range(NK)]
            cos2 = tab.tile([128, T], BF16, tag="cos")
            sin2 = tab.tile([128, T], BF16, tag="sin")
            tri = tab.tile([128, 128], BF16, tag="tri")
            qT = [qkp.tile([128, T], BF16, tag=f"q{p}", name=f"qT{p}")
                  for p in range(2)]
            kT = [qkp.tile([128, T], BF16, tag=f"k{p}", name=f"kTt{p}")
                  for p in range(2)]
            # per pair: 16 k-blocks x [2 heads x (64 v | 1 ones)]
            vap = [vap_p.tile([128, 16 * 130], BF16, tag=f"va{p}",
                              name=f"vap{p}") for p in range(2)]
            oT = [otp.tile([128, T], BF16, tag=f"o{p}", name=f"oT{p}")
                  for p in range(2)]
            wo_sb = [wop.tile([128, D], BF16, tag=f"wo{p}", name=f"wo{p}")
                     for p in range(2)]

            # ---- DMA issue (w on scalar queue, tables on vector queue, x on
            # sync queue t-chunk-major so chunk (kt, 0) lands first).
            for kt in range(NK):
                nc.scalar.dma_start(
                    out=w_sb[kt][:],
                    in_=d_w[kt * 128:(kt + 1) * 128, :].bitcast(F32R))
            nc.scalar.dma_start(out=cos2[:], in_=d_cos[:])
            nc.scalar.dma_start(out=sin2[:], in_=d_sin[:])
            nc.scalar.dma_start(out=tri[:], in_=d_tri[:])
            for p in range(2):
                nc.scalar.dma_start(
                    out=wo_sb[p][:], in_=d_wo[p * 128:(p + 1) * 128, :])
                # softmax-denominator ones columns
                nc.gpsimd.memset(vap[p][:, 64:16 * 130:65], 1.0)
            for tcc in range(NTC):
                for kt in range(NK):
                    nc.sync.dma_start(
                        out=x_sb[kt][:, tcc * 512:(tcc + 1) * 512],
                        in_=d_xT[kt * 128:(kt + 1) * 128,
                                 tcc * 512:(tcc + 1) * 512].bitcast(F32R))

            # ---- helpers ------------------------------------------------
            def mm_ab(t_ab, t_cd, tcc, kt):
                """q01|k01 into t_ab halves, q23|k23 into t_cd halves."""
                c0 = tcc * 512
                for half, wc in ((t_ab, 0), (t_cd, 2)):
                    for i in range(2):
                        nc.tensor.matmul(
                            half[:, i * 512:(i + 1) * 512],
                            w_sb[kt][:, (wc + i) * 128:(wc + i + 1) * 128],
                            x_sb[kt][:, c0:c0 + 512],
                            start=(kt == 0), stop=(kt == NK - 1))

            def mm_v(t_v, tcc, kt):
                """v for 4 t-blocks: x chunk stationary, w_v moving."""
                for tb in range(4):
                    t0 = tcc * 512 + tb * 128
                    nc.tensor.matmul(
                        t_v[:, tb * 256:(tb + 1) * 256],
                        x_sb[kt][:, t0:t0 + 128],
                        w_sb[kt][:, 4 * 128:6 * 128],
                        start=(kt == 0), stop=(kt == NK - 1))

            def emit_rope(ps_ab, p, tcc, which):
                """Drain a q|k PSUM pair-tile through RoPE into bf16 SBUF."""
                cs = slice(tcc * 512, tcc * 512 + 512)
                for half, dst in ((0, qT[p]), (1, kT[p])):
                    src = ps_ab[:, half * 512:(half + 1) * 512]
                    qsh = rsc.tile([128, 512], BF16, tag="qsh",
                                   name=f"qsh{which}_{half}")
                    tcs = rsc.tile([128, 512], BF16, tag="tcs",
                                   name=f"tcs{which}_{half}")
                    nc.vector.stream_shuffle(qsh[:], src, SWAP_MASK)
                    nc.gpsimd.tensor_tensor(out=tcs[:], in0=src,
                                            in1=cos2[:, cs], op=ALU.mult)
                    nc.vector.tensor_tensor(out=qsh[:], in0=qsh[:],
                                            in1=sin2[:, cs], op=ALU.mult)
                    nc.vector.tensor_tensor(out=dst[:, cs], in0=qsh[:],
                                            in1=tcs[:], op=ALU.add)

            def emit_vdrain(ps_v, tcc):
                """PSUM [128, 4*(4*64)] -> vap strided (65-col head slots)."""
                v4 = ps_v[:].rearrange("a (tb h d) -> a tb h d", tb=4, h=4)
                for p in range(2):
                    o4 = vap[p][:].rearrange("a (kb hh e) -> a kb hh e",
                                             kb=16, hh=2)
                    nc.gpsimd.tensor_copy(
                        out=o4[:, tcc * 4:(tcc + 1) * 4, :, 0:64],
                        in_=v4[:, :, 2 * p:2 * p + 2, :])

            # ---- phase A, t-chunk 0: fully DMA-paced, 3 PSUM tiles ------
            psA = tc.alloc_tile_pool(name="psA", bufs=2, space="PSUM")
            psC0 = tc.alloc_tile_pool(name="psC0", bufs=1, space="PSUM",
                                      side="right")
            tA = psA.tile([128, 1024], F32, tag="pa", name="pA0")
            tB = psA.tile([128, 1024], F32, tag="pa", name="pB0")
            tC = psC0.tile([128, 1024], F32, tag="pc", name="pC0")
            for kt in range(NK):
                mm_ab(tA, tB, 0, kt)
                mm_v(tC, 0, kt)
            emit_rope(tA, 0, 0, "A0")
            emit_rope(tB, 1, 0, "B0")
            emit_vdrain(tC, 0)
            psC0.release()

            psS = tc.alloc_tile_pool(name="psS", bufs=1, space="PSUM",
                                     side="right")
            psV = tc.alloc_tile_pool(name="psV", bufs=1, space="PSUM",
                                     side="right")

            # ---- lazy filler units: remaining projection, then oproj ----
            def qkv_units():
                for tcc in range(1, NTC):
                    t_ab = psA.tile([128, 1024], F32, tag="pa",
                                    name=f"pA{tcc}")
                    t_cd = psA.tile([128, 1024], F32, tag="pa",
                                    name=f"pB{tcc}")
                    for kt in range(NK):
                        mm_ab(t_ab, t_cd, tcc, kt)
                        yield
                    emit_rope(t_ab, 0, tcc, f"A{tcc}")
                    t_v = psA.tile([128, 1024], F32, tag="pa",
                                   name=f"pC{tcc}")
                    for kt in range(NK):
                        mm_v(t_v, tcc, kt)
                        yield
                    emit_rope(t_cd, 1, tcc, f"B{tcc}")
                    emit_vdrain(t_v, tcc)

            state = {"gen": qkv_units()}

            def fill(n):
                g = state["gen"]
                if g is None:
                    return
                for _ in range(n):
                    try:
                        next(g)
                    except StopIteration:
                        state["gen"] = None
                        return

            def drain_fill():
                fill(1 << 30)

            opj = {"psD": None}

            def _psD():
                if opj["psD"] is None:
                    opj["psD"] = tc.alloc_tile_pool(name="psD", bufs=2,
                                                    space="PSUM")
                return opj["psD"]

            def emit_oproj_n(si, n):
                psD = _psD()
                q0 = si * 512
                pD = psD.tile([128, 512], F32, tag="pd", name=f"pD{si}_{n}")
                for p in range(2):
                    nc.tensor.matmul(
                        pD[:], wo_sb[p][:, n * 128:(n + 1) * 128],
                        oT[p][:, q0:q0 + 512],
                        start=(p == 0), stop=(p == 1))
                fo = fop.tile([128, 512], BF16, tag="fo", name=f"fo{si}_{n}")
                fo_eng = nc.vector if n % 2 else nc.gpsimd
                fo_eng.tensor_copy(out=fo[:], in_=pD[:])
                nc.sync.dma_start(
                    out=d_out[n * 128:(n + 1) * 128, q0:q0 + 512],
                    in_=fo[:])

            def oproj_units():
                # strips 0..2 only: strip 3's oproj depends on strip 3's
                # normalize, which is emitted after these fillers — including
                # it here would wedge the in-order PE queue.
                for si in range(3):
                    for n in range(8):
                        emit_oproj_n(si, n)
                        yield

            # ---- attention strip (pairs serial) -------------------------
            def emit_strip_pair(si, p):
                q0 = 512 * si
                kbmax = 4 * (si + 1)
                av = [psV.tile([65, 512], F32, tag=f"av{hl}",
                               name=f"av{si}_{p}_{hl}") for hl in range(2)]
                for kb in range(kbmax):
                    o = max(0, 128 * kb - q0)
                    L = 512 - o
                    sps = psS.tile([128, 1024], F32, tag="ps",
                                   name=f"sps{si}_{p}_{kb}")
                    for hl in range(2):
                        hb = 64 * hl
                        nc.tensor.matmul(
                            sps[:, 512 * hl:512 * hl + L],
                            kT[p][hb:hb + 64, kb * 128:(kb + 1) * 128],
                            qT[p][hb:hb + 64, q0 + o:q0 + 512],
                            start=True, stop=True)
                    ptb = ptp.tile([128, 1024], BF16, tag="ptb",
                                   name=f"ptb{si}_{p}_{kb}")
                    sps3 = sps[:].rearrange("a (h q) -> a h q", h=2)
                    ptb3 = ptb[:].rearrange("a (h q) -> a h q", h=2)
                    nc.scalar.activation(ptb3[:, :, 0:L], sps3[:, :, o:512],
                                         AF.Exp, scale=SCALE)
                    if 128 * kb >= q0:
                        # diagonal block: zero cols j < partition (q < k)
                        nc.gpsimd.affine_select(
                            ptb[:, 0:128], ptb[:, 0:128],
                            pattern=[[1, 128]], compare_op=ALU.is_ge,
                            fill=0.0, base=0, channel_multiplier=-1)
                        nc.vector.tensor_tensor(
                            out=ptb[:, 512:640], in0=ptb[:, 512:640],
                            in1=tri[:], op=ALU.mult)
                    fill(1)
                    for hl in range(2):
                        nc.tensor.matmul(
                            av[hl][:, o:512],
                            vap[p][:, kb * 130 + hl * 65:
                                   kb * 130 + hl * 65 + 65],
                            ptb[:, 512 * hl:512 * hl + L],
                            start=(kb == 0), stop=(kb == kbmax - 1),
                            skip_group_check=True)
                    fill(1)
                for hl in range(2):
                    r_sb = rp.tile([1, 512], F32, tag="r",
                                   name=f"rsb{si}_{p}_{hl}")
                    nc.vector.reciprocal(r_sb[:], av[hl][64:65, :])
                    rb = rp.tile([64, 512], F32, tag="rb",
                                 name=f"rbb{si}_{p}_{hl}")
                    nc.gpsimd.partition_broadcast(rb[:], r_sb[:])
                    nc.gpsimd.tensor_tensor(
                        out=oT[p][64 * hl:64 * hl + 64, q0:q0 + 512],
                        in0=av[hl][0:64, :], in1=rb[:], op=ALU.mult)

            with nc.named_scope("attn"):
                for si in range(3):
                    emit_strip_pair(si, 0)
                    emit_strip_pair(si, 1)
                drain_fill()
                psA.release()
                state["gen"] = oproj_units()
                emit_strip_pair(3, 0)
                emit_strip_pair(3, 1)
                drain_fill()
                for n in range(8):
                    emit_oproj_n(3, n)
                opj["psD"].release()

            psV.release()
            psS.release()

            if dbg:
                nc.sync.dma_start(out=d_dbg_q0[:], in_=qT[0][:])
                nc.sync.dma_start(out=d_dbg_k0[:], in_=kT[0][:])
                nc.sync.dma_start(out=d_dbg_va0[:], in_=vap[0][:])
                nc.sync.dma_start(out=d_dbg_o0[:], in_=oT[0][:])

    nc.compile()
    return nc


_NC_CACHE = None


def _get_program():
    global _NC_CACHE
    if _NC_CACHE is None:
        _NC_CACHE = _build_program()
    return _NC_CACHE


def _rope_tables():
    inv_freq = 1.0 / (10000.0 ** (np.arange(0, HD, 2, dtype=np.float32) / HD))
    freqs = np.outer(np.arange(T, dtype=np.float32), inv_freq)  # [T, 32]
    emb = np.concatenate([freqs, freqs], axis=-1)               # [T, 64]
    return np.cos(emb), np.sin(emb)


def _host_prep(x, w_qkv, w_out):
    bf = ml_dtypes.bfloat16
    cos, sin = _rope_tables()          # [T, 64], original hd order
    cosP = np.ascontiguousarray(cos.T[PI, :])                   # [64, T]
    sinP = sin.T[PI, :].copy()                                  # [64, T]
    sinP[0::2, :] *= -1.0                                       # sign baked in
    cos2 = np.ascontiguousarray(np.vstack([cosP, cosP])).astype(bf)
    sin2 = np.ascontiguousarray(np.vstack([sinP, sinP])).astype(bf)
    tri = np.triu(np.ones((128, 128), dtype=np.float32)).astype(bf)

    in_maps = []
    for core in range(NCORES):
        b = core // GROUPS
        h0 = (core % GROUPS) * HPC
        xT = np.ascontiguousarray(x[b].T)                       # [D, T]
        cols = []
        for kind in range(2):                                   # q, k pairs
            for pr in range(2):
                for hh in range(2):
                    h = h0 + 2 * pr + hh
                    wc = w_qkv[:, kind * D + h * HD:kind * D + (h + 1) * HD]
                    cols.append(wc[:, PI])
        # reorder to q01|k01|q23|k23
        cols = [cols[0], cols[1], cols[4], cols[5],
                cols[2], cols[3], cols[6], cols[7]]
        cols = [np.concatenate(cols[0:2], axis=1),
                np.concatenate(cols[2:4], axis=1),
                np.concatenate(cols[4:6], axis=1),
                np.concatenate(cols[6:8], axis=1),
                w_qkv[:, 2 * D + h0 * HD:2 * D + (h0 + HPC) * HD]]
        w_cat = np.ascontiguousarray(np.concatenate(cols, axis=1),
                                     dtype=np.float32)          # [D, 768]
        w_o = np.ascontiguousarray(
            w_out[h0 * HD:(h0 + HPC) * HD, :]).astype(bf)       # [256, D]
        in_maps.append({
            "xT": xT.astype(np.float32, copy=False),
            "w_cat": w_cat,
            "w_o": w_o,
            "cos2": cos2,
            "sin2": sin2,
            "tri": tri,
        })
    return in_maps


def kernel(x, w_qkv, w_out):
    x = np.asarray(x, dtype=np.float32)
    w_qkv = np.asarray(w_qkv, dtype=np.float32)
    w_out = np.asarray(w_out, dtype=np.float32)
    nc = _get_program()
    in_maps = _host_prep(x, w_qkv, w_out)
    res = run_bass_kernel_spmd(nc, in_maps, list(range(NCORES)), trace=False)
    out = np.zeros((B, T, D), dtype=np.float32)
    for core in range(NCORES):
        b = core // GROUPS
        out[b] += res.results[core]["outp"].T.astype(np.float32)
    return out


# revision 15
# speedup vs baseline: 1.3423x; 1.3423x over previous
# Causal self-attention (B=2, T=2048, D=1024, H=16, HD=64) with RoPE on 8 TRN2
# cores. Data-parallel over batch (2 groups of 4 cores), tensor-parallel over
# heads within a group (4 heads = 2 pairs per core).
#
# Schedule (single pass, engines balanced):
#  - x arrives t-chunk-major ([kt, 512-col chunk] DMAs) so the qkv^T projection
#    pipelines behind the x load instead of stalling on it.
#  - q/k projected per head-pair into PSUM, RoPE'd (DVE+Pool) into bf16 SBUF;
#    v projected directly in [t, hd] layout (x chunk as the stationary
#    operand), so no PE transposes are needed for AV.
#  - attention strip si (512 q's) runs as soon as its q/k/v t-chunks exist,
#    interleaved into the remaining projection matmuls as PE filler: S^T
#    (bf16) -> exp on the Scalar engine (only exp lives there) -> causal
#    triangle mask (Pool affine_select / DVE mask-multiply) -> AV with an
#    augmented ones-column producing the softmax denominator.
#  - out-projection (row-sharded, partial [D, T] per core) per strip, woven
#    into the last strip's exp gaps; host sums 4 partials per batch.
import sys
import os

sys.path.insert(0, "/opt/trn_rl_repo")

import numpy as np
import ml_dtypes

import concourse.bass as bass  # noqa: F401
import concourse.mybir as mybir
from concourse import bacc
from concourse.tile import TileContext
from concourse.bass_utils import run_bass_kernel_spmd
from contextlib import ExitStack

F32 = mybir.dt.float32
F32R = mybir.dt.float32r
BF16 = mybir.dt.bfloat16
AF = mybir.ActivationFunctionType
ALU = mybir.AluOpType

B, T, D = 2, 2048, 1024
H, HD = 16, 64
NCORES = 8
GROUPS = NCORES // B          # cores per batch = 4
HPC = H // GROUPS             # heads per core = 4
NK = D // 128                 # contraction tiles = 8
NTC = T // 512                # t-chunks = 4
SCALE = HD ** -0.5

# hd interleave: new row 2j <- orig j, new row 2j+1 <- orig j+32 so the
# rotate-half partner of every row is its neighbour (swappable by a 32-lane
# stream shuffle).
PI = np.empty(HD, dtype=np.int64)
PI[0::2] = np.arange(32)
PI[1::2] = np.arange(32, 64)

SWAP_MASK = []
for _i in range(16):
    SWAP_MASK += [2 * _i + 1, 2 * _i]


def _build_program():
    nc = bacc.Bacc("TRN2", target_bir_lowering=False, debug=False,
                   num_devices=NCORES)
    d_xT = nc.dram_tensor("xT", [D, T], F32, kind="ExternalInput").ap()
    # cols: q01|k01|q23|k23 (PI-interleaved, 128 each) then v0..v3 (plain, 256)
    d_w = nc.dram_tensor("w_cat", [D, 6 * 128], F32, kind="ExternalInput").ap()
    d_wo = nc.dram_tensor("w_o", [2 * 128, D], BF16, kind="ExternalInput").ap()
    d_cos = nc.dram_tensor("cos2", [128, T], BF16, kind="ExternalInput").ap()
    d_sin = nc.dram_tensor("sin2", [128, T], BF16, kind="ExternalInput").ap()
    d_tri = nc.dram_tensor("tri", [128, 128], BF16, kind="ExternalInput").ap()
    d_out = nc.dram_tensor("outp", [D, T], BF16, kind="ExternalOutput").ap()
    dbg = bool(int(os.environ.get("KDEBUG", "0")))
    if dbg:
        d_dbg_q0 = nc.dram_tensor("dbg_q0", [128, T], BF16,
                                  kind="ExternalOutput").ap()
        d_dbg_k0 = nc.dram_tensor("dbg_k0", [128, T], BF16,
                                  kind="ExternalOutput").ap()
        d_dbg_va0 = nc.dram_tensor("dbg_va0", [128, 16 * 130], BF16,
                                   kind="ExternalOutput").ap()
        d_dbg_o0 = nc.dram_tensor("dbg_o0", [128, T], BF16,
                                  kind="ExternalOutput").ap()

    with TileContext(nc) as tc, nc.allow_low_precision(reason="bf16 attn"):
        with ExitStack() as root:
            xp = root.enter_context(tc.tile_pool(name="xp", bufs=1))
            wp = root.enter_context(tc.tile_pool(name="wp", bufs=1))
            tab = root.enter_context(tc.tile_pool(name="tab", bufs=1))
            qkp = root.enter_context(tc.tile_pool(name="qkp", bufs=1))
            vap_p = root.enter_context(tc.tile_pool(name="vap", bufs=1))
            otp = root.enter_context(tc.tile_pool(name="otp", bufs=1))
            wop = root.enter_context(tc.tile_pool(name="wop", bufs=1))
            rsc = root.enter_context(tc.tile_pool(name="rsc", bufs=3))
            ptp = root.enter_context(tc.tile_pool(name="ptp", bufs=6))
            rp = root.enter_context(tc.tile_pool(name="rp", bufs=2))
            fop = root.enter_context(tc.tile_pool(name="fop", bufs=4))

            x_sb = [xp.tile([128, T], F32R, tag=f"x{kt}", name=f"xsb{kt}")
                    for kt in range(NK)]
            w_sb = [wp.tile([128, 6 * 128], F32R, tag=f"w{kt}",
                            name=f"wsb{kt}") for kt in range(NK)]
            cos2 = tab.tile([128, T], BF16, tag="cos")
            sin2 = tab.tile([128, T], BF16, tag="sin")
            tri = tab.tile([128, 128], BF16, tag="tri")
            qT = [qkp.tile([128, T], BF16, tag=f"q{p}", name=f"qT{p}")
                  for p in range(2)]
            kT = [qkp.tile([128, T], BF16, tag=f"k{p}", name=f"kTt{p}")
                  for p in range(2)]
            # per pair: 16 k-blocks x [2 heads x (64 v | 1 ones)]
            vap = [vap_p.tile([128, 16 * 130], BF16, tag=f"va{p}",
                              name=f"vap{p}") for p in range(2)]
            oT = [otp.tile([128, T], BF16, tag=f"o{p}", name=f"oT{p}")
                  for p in range(2)]
            wo_sb = [wop.tile([128, D], BF16, tag=f"wo{p}", name=f"wo{p}")
                     for p in range(2)]

            # ---- DMA issue (w on scalar queue, tables on vector queue, x on
            # sync queue t-chunk-major so chunk (kt, 0) lands first).
            for kt in range(NK):
                nc.scalar.dma_start(
                    out=w_sb[kt][:],
                    in_=d_w[kt * 128:(kt + 1) * 128, :].bitcast(F32R))
            nc.scalar.dma_start(out=cos2[:], in_=d_cos[:])
            nc.scalar.dma_start(out=sin2[:], in_=d_sin[:])
            nc.scalar.dma_start(out=tri[:], in_=d_tri[:])
            for p in range(2):
                nc.scalar.dma_start(
                    out=wo_sb[p][:], in_=d_wo[p * 128:(p + 1) * 128, :])
                # softmax-denominator ones columns
                nc.gpsimd.memset(vap[p][:, 64:16 * 130:65], 1.0)
            for tcc in range(NTC):
                for kt in range(NK):
                    nc.sync.dma_start(
                        out=x_sb[kt][:, tcc * 512:(tcc + 1) * 512],
                        in_=d_xT[kt * 128:(kt + 1) * 128,
                                 tcc * 512:(tcc + 1) * 512].bitcast(F32R))

            # ---- helpers ------------------------------------------------
            def mm_ab(t_ab, t_cd, tcc, kt):
                """q01|k01 into t_ab halves, q23|k23 into t_cd halves."""
                c0 = tcc * 512
                for half, wc in ((t_ab, 0), (t_cd, 2)):
                    for i in range(2):
                        nc.tensor.matmul(
                            half[:, i * 512:(i + 1) * 512],
                            w_sb[kt][:, (wc + i) * 128:(wc + i + 1) * 128],
                            x_sb[kt][:, c0:c0 + 512],
                            start=(kt == 0), stop=(kt == NK - 1))

            def mm_v(t_v, tcc, kt):
                """v for 4 t-blocks: x chunk stationary, w_v moving."""
                for tb in range(4):
                    t0 = tcc * 512 + tb * 128
                    nc.tensor.matmul(
                        t_v[:, tb * 256:(tb + 1) * 256],
                        x_sb[kt][:, t0:t0 + 128],
                        w_sb[kt][:, 4 * 128:6 * 128],
                        start=(kt == 0), stop=(kt == NK - 1))

            def emit_rope(ps_ab, p, tcc, which):
                """Drain a q|k PSUM pair-tile through RoPE into bf16 SBUF."""
                cs = slice(tcc * 512, tcc * 512 + 512)
                for half, dst in ((0, qT[p]), (1, kT[p])):
                    src = ps_ab[:, half * 512:(half + 1) * 512]
                    qsh = rsc.tile([128, 512], BF16, tag="qsh",
                                   name=f"qsh{which}_{half}")
                    tcs = rsc.tile([128, 512], BF16, tag="tcs",
                                   name=f"tcs{which}_{half}")
                    nc.vector.stream_shuffle(qsh[:], src, SWAP_MASK)
                    nc.vector.tensor_tensor(out=tcs[:], in0=src,
                                            in1=cos2[:, cs], op=ALU.mult)
                    nc.vector.tensor_tensor(out=qsh[:], in0=qsh[:],
                                            in1=sin2[:, cs], op=ALU.mult)
                    nc.vector.tensor_tensor(out=dst[:, cs], in0=qsh[:],
                                            in1=tcs[:], op=ALU.add)

            def emit_vdrain(ps_v, tcc):
                """PSUM [128, 4*(4*64)] -> vap strided (65-col head slots)."""
                v4 = ps_v[:].rearrange("a (tb h d) -> a tb h d", tb=4, h=4)
                for p in range(2):
                    o4 = vap[p][:].rearrange("a (kb hh e) -> a kb hh e",
                                             kb=16, hh=2)
                    nc.gpsimd.tensor_copy(
                        out=o4[:, tcc * 4:(tcc + 1) * 4, :, 0:64],
                        in_=v4[:, :, 2 * p:2 * p + 2, :])

            # ---- phase A, t-chunk 0: fully DMA-paced, 3 PSUM tiles ------
            psA = tc.alloc_tile_pool(name="psA", bufs=2, space="PSUM")
            psC0 = tc.alloc_tile_pool(name="psC0", bufs=1, space="PSUM",
                                      side="right")
            tA = psA.tile([128, 1024], F32, tag="pa", name="pA0")
            tB = psA.tile([128, 1024], F32, tag="pa", name="pB0")
            tC = psC0.tile([128, 1024], F32, tag="pc", name="pC0")
            for kt in range(NK):
                mm_ab(tA, tB, 0, kt)
                mm_v(tC, 0, kt)
            emit_rope(tA, 0, 0, "A0")
            emit_rope(tB, 1, 0, "B0")
            emit_vdrain(tC, 0)
            psC0.release()

            psS = tc.alloc_tile_pool(name="psS", bufs=1, space="PSUM",
                                     side="right")
            psV = tc.alloc_tile_pool(name="psV", bufs=1, space="PSUM",
                                     side="right")
            pools = {"S": psS, "V": psV}

            # ---- lazy filler units: remaining projection, then oproj ----
            def qkv_units():
                for tcc in range(1, NTC):
                    t_ab = psA.tile([128, 1024], F32, tag="pa",
                                    name=f"pA{tcc}")
                    t_cd = psA.tile([128, 1024], F32, tag="pa",
                                    name=f"pB{tcc}")
                    for kt in range(NK):
                        mm_ab(t_ab, t_cd, tcc, kt)
                        yield
                    emit_rope(t_ab, 0, tcc, f"A{tcc}")
                    t_v = psA.tile([128, 1024], F32, tag="pa",
                                   name=f"pC{tcc}")
                    for kt in range(NK):
                        mm_v(t_v, tcc, kt)
                        yield
                    emit_rope(t_cd, 1, tcc, f"B{tcc}")
                    emit_vdrain(t_v, tcc)

            state = {"gen": qkv_units()}

            def fill(n):
                g = state["gen"]
                if g is None:
                    return
                for _ in range(n):
                    try:
                        next(g)
                    except StopIteration:
                        state["gen"] = None
                        return

            def drain_fill():
                fill(1 << 30)

            def emit_oproj_n(psD, si, n, via_sbuf=False):
                q0 = si * 512
                pD = psD.tile([128, 512], F32, tag="pd", name=f"pD{si}_{n}")
                for p in range(2):
                    nc.tensor.matmul(
                        pD[:], wo_sb[p][:, n * 128:(n + 1) * 128],
                        oT[p][:, q0:q0 + 512],
                        start=(p == 0), stop=(p == 1))
                dst = d_out[n * 128:(n + 1) * 128, q0:q0 + 512]
                fo = fop.tile([128, 512], BF16, tag="fo", name=f"fo{si}_{n}")
                (nc.vector if via_sbuf and n % 2 else nc.gpsimd).tensor_copy(
                    out=fo[:], in_=pD[:])
                nc.sync.dma_start(out=dst, in_=fo[:])

            def oproj_units(psD, strips):
                # strip 3's oproj must NOT be a filler: it depends on strip
                # 3's normalize, emitted after the fillers — including it
                # would wedge the in-order PE queue.
                for si in strips:
                    for n in range(8):
                        emit_oproj_n(psD, si, n)
                        yield

            # ---- attention strip (pairs serial) -------------------------
            def emit_strip_pair(si, p):
                q0 = 512 * si
                kbmax = 4 * (si + 1)
                av = [pools["V"].tile([65, 512], F32, tag=f"av{hl}",
                                      name=f"av{si}_{p}_{hl}")
                      for hl in range(2)]
                for kb in range(kbmax):
                    o = max(0, 128 * kb - q0)
                    L = 512 - o
                    sps = pools["S"].tile([128, 1024], F32, tag="ps",
                                          name=f"sps{si}_{p}_{kb}")
                    for hl in range(2):
                        hb = 64 * hl
                        nc.tensor.matmul(
                            sps[:, 512 * hl:512 * hl + L],
                            kT[p][hb:hb + 64, kb * 128:(kb + 1) * 128],
                            qT[p][hb:hb + 64, q0 + o:q0 + 512],
                            start=True, stop=True)
                    ptb = ptp.tile([128, 1024], BF16, tag="ptb",
                                   name=f"ptb{si}_{p}_{kb}")
                    sps3 = sps[:].rearrange("a (h q) -> a h q", h=2)
                    ptb3 = ptb[:].rearrange("a (h q) -> a h q", h=2)
                    nc.scalar.activation(ptb3[:, :, 0:L], sps3[:, :, o:512],
                                         AF.Exp, scale=SCALE)
                    if 128 * kb >= q0:
                        # diagonal block: zero cols j < partition (q < k)
                        nc.gpsimd.affine_select(
                            ptb[:, 0:128], ptb[:, 0:128],
                            pattern=[[1, 128]], compare_op=ALU.is_ge,
                            fill=0.0, base=0, channel_multiplier=-1)
                        nc.vector.tensor_tensor(
                            out=ptb[:, 512:640], in0=ptb[:, 512:640],
                            in1=tri[:], op=ALU.mult)
                    fill(1)
                    for hl in range(2):
                        nc.tensor.matmul(
                            av[hl][:, o:512],
                            vap[p][:, kb * 130 + hl * 65:
                                   kb * 130 + hl * 65 + 65],
                            ptb[:, 512 * hl:512 * hl + L],
                            start=(kb == 0), stop=(kb == kbmax - 1),
                            skip_group_check=True)
                    fill(1)
                for hl in range(2):
                    r_sb = rp.tile([1, 512], F32, tag="r",
                                   name=f"rsb{si}_{p}_{hl}")
                    nc.vector.reciprocal(r_sb[:], av[hl][64:65, :])
                    rb = rp.tile([64, 512], F32, tag="rb",
                                 name=f"rbb{si}_{p}_{hl}")
                    nc.gpsimd.partition_broadcast(rb[:], r_sb[:])
                    nc.vector.tensor_tensor(
                        out=oT[p][64 * hl:64 * hl + 64, q0:q0 + 512],
                        in0=av[hl][0:64, :], in1=rb[:], op=ALU.mult)

            with nc.named_scope("attn"):
                # strips 0-1: qkv projection as PE filler, psS single-buffered
                for si in range(2):
                    emit_strip_pair(si, 0)
                    emit_strip_pair(si, 1)
                drain_fill()
                # re-plumb PSUM: double-buffered S tiles (exp overlaps next
                # S), av accumulators, oproj accumulators — exactly 8 banks.
                psV.release()
                psS.release()
                psA.release()
                psS2 = tc.alloc_tile_pool(name="psS2", bufs=2, space="PSUM")
                psV2 = tc.alloc_tile_pool(name="psV2", bufs=1, space="PSUM",
                                          side="right")
                psD = tc.alloc_tile_pool(name="psD", bufs=2, space="PSUM",
                                         side="right")
                pools["S"], pools["V"] = psS2, psV2
                state["gen"] = oproj_units(psD, [0, 1])
                emit_strip_pair(2, 0)
                emit_strip_pair(2, 1)
                drain_fill()
                state["gen"] = oproj_units(psD, [2])
                emit_strip_pair(3, 0)
                emit_strip_pair(3, 1)
                drain_fill()
                for n in range(8):
                    emit_oproj_n(psD, 3, n, via_sbuf=True)
                psD.release()
                psV2.release()
                psS2.release()

            if dbg:
                nc.sync.dma_start(out=d_dbg_q0[:], in_=qT[0][:])
                nc.sync.dma_start(out=d_dbg_k0[:], in_=kT[0][:])
                nc.sync.dma_start(out=d_dbg_va0[:], in_=vap[0][:])
                nc.sync.dma_start(out=d_dbg_o0[:], in_=oT[0][:])

    nc.compile()
    return nc


_NC_CACHE = None


def _get_program():
    global _NC_CACHE
    if _NC_CACHE is None:
        _NC_CACHE = _build_program()
    return _NC_CACHE


def _rope_tables():
    inv_freq = 1.0 / (10000.0 ** (np.arange(0, HD, 2, dtype=np.float32) / HD))
    freqs = np.outer(np.arange(T, dtype=np.float32), inv_freq)  # [T, 32]
    emb = np.concatenate([freqs, freqs], axis=-1)               # [T, 64]
    return np.cos(emb), np.sin(emb)


def _host_prep(x, w_qkv, w_out):
    bf = ml_dtypes.bfloat16
    cos, sin = _rope_tables()          # [T, 64], original hd order
    cosP = np.ascontiguousarray(cos.T[PI, :])                   # [64, T]
    sinP = sin.T[PI, :].copy()                                  # [64, T]
    sinP[0::2, :] *= -1.0                                       # sign baked in
    cos2 = np.ascontiguousarray(np.vstack([cosP, cosP])).astype(bf)
    sin2 = np.ascontiguousarray(np.vstack([sinP, sinP])).astype(bf)
    tri = np.triu(np.ones((128, 128), dtype=np.float32)).astype(bf)

    in_maps = []
    for core in range(NCORES):
        b = core // GROUPS
        h0 = (core % GROUPS) * HPC
        xT = np.ascontiguousarray(x[b].T)                       # [D, T]
        cols = []
        for kind in range(2):                                   # q, k pairs
            for pr in range(2):
                for hh in range(2):
                    h = h0 + 2 * pr + hh
                    wc = w_qkv[:, kind * D + h * HD:kind * D + (h + 1) * HD]
                    cols.append(wc[:, PI])
        # reorder to q01|k01|q23|k23
        cols = [cols[0], cols[1], cols[4], cols[5],
                cols[2], cols[3], cols[6], cols[7]]
        cols = [np.concatenate(cols[0:2], axis=1),
                np.concatenate(cols[2:4], axis=1),
                np.concatenate(cols[4:6], axis=1),
                np.concatenate(cols[6:8], axis=1),
                w_qkv[:, 2 * D + h0 * HD:2 * D + (h0 + HPC) * HD]]
        w_cat = np.ascontiguousarray(np.concatenate(cols, axis=1),
                                     dtype=np.float32)          # [D, 768]
        w_o = np.ascontiguousarray(
            w_out[h0 * HD:(h0 + HPC) * HD, :]).astype(bf)       # [256, D]
        in_maps.append({
            "xT": xT.astype(np.float32, copy=False),
            "w_cat": w_cat,
            "w_o": w_o,
            "cos2": cos2,
            "sin2": sin2,
            "tri": tri,
        })
    return in_maps


def kernel(x, w_qkv, w_out):
    x = np.asarray(x, dtype=np.float32)
    w_qkv = np.asarray(w_qkv, dtype=np.float32)
    w_out = np.asarray(w_out, dtype=np.float32)
    nc = _get_program()
    in_maps = _host_prep(x, w_qkv, w_out)
    res = run_bass_kernel_spmd(nc, in_maps, list(range(NCORES)), trace=False)
    out = np.zeros((B, T, D), dtype=np.float32)
    for core in range(NCORES):
        b = core // GROUPS
        out[b] += res.results[core]["outp"].T.astype(np.float32)
    return out
